# revision 1
# baseline (speedup 1.0000x reference)
"""MoLA adapter (MoE-of-LoRA) Trainium2 kernel.

out = x @ W_base.T + b_base
      + sum_{e in top2(router(x))} softmax_gate_e * (x @ A_e.T @ B_e.T) * (alpha/r)

Strategy (8 NeuronCores, data-parallel over tokens; 1024 tokens/core):
  - Host pre-transposes x (-> xT [D, 1024] per core) and W_base (-> wT [D, O]),
    packs LoRA-A as rc [D, 64] (f32r), router as rcl [D, 8] (f32), and
    (SCALE*B_cat | b_base) as bc [65, O] (f32r).
  - hT [64+1, tok]: A.T @ x with A stationary, x moving at N=512 (f32r full
    rate); row 64 is ones (carries the bias through the combine matmul).
  - logits: token-major [128, 8] per tile via N=8 fp32 matmuls (exact top-k
    selection; f32r logits flip near-tie experts), 8 groups into one PSUM bank.
  - top-2 + softmax-of-2 gates: one batched DVE/ACT chain over all 8 token
    tiles at once ([128, 8 tiles x 8 experts]), then per tile a broadcast
    gate-replicate [128, 65], PE transpose, and whT = hT * gatesT (f32->f32r).
  - y tile [128, 512] PSUM accumulates 16 base matmuls (xT_k.T @ wT_k, f32r,
    1 col/cycle) plus one K=65 combine matmul whT.T @ bc (LoRA update + bias).
  - float32r is the full-rate fp32 PE path (1 col/cycle at N>=256 vs 4 for
    fp32): measured l2 rel err 1.5e-4 on a K=2048 matmul, 16x better than
    bf16, at bf16 speed.
  - DMA: ~33 MB/core at ~360 GB/s is ~92 us vs ~127 us PE busy. Loads go on
    the SP HWDGE ring in 0.5-2 MB chunks (x staggered small-first so the PE
    starts at ~2 us; W0 quarters interleaved so base groups chase the
    stream); stores and small constants go on the ACT HWDGE ring. 5 PSUM
    banks of open base-accumulation groups hide most of the x+W0 prefix.
"""

import numpy as np

import concourse.bass as bass
import concourse.mybir as mybir
import concourse.tile as tile
from concourse import bacc
from concourse.bass_utils import run_bass_kernel_spmd
from concourse.masks import make_identity

# problem dims (hardcoded per contract)
B, S, D, O = 4, 2048, 2048, 2048
E, R, TOPK = 8, 8, 2
SCALE = 16.0 / R  # alpha / r
NCORES = 8
NTOK = B * S
TOK = NTOK // NCORES        # tokens per core = 1024
NT = TOK // 128             # 8 token tiles per core
KC = D // 128               # 16 contraction chunks
OCH = 512
NOC = O // OCH              # 4 output chunks
ER = E * R                  # 64
RC = E + ER                 # 72: router logits + all-expert h

F32 = mybir.dt.float32
F32R = mybir.dt.float32r

_CACHE = {}


def _build_program(use_f32r=True):
    key = ("prog", use_f32r)
    if key in _CACHE:
        return _CACHE[key]
    nc = bacc.Bacc("TRN2", target_bir_lowering=False, debug=False)
    dt_mm = F32R if use_f32r else F32
    TC2 = TOK // 512            # 2 chunks of 512 tokens for the prT matmul
    xT = nc.dram_tensor("xT", [D, TOK], dt_mm, kind="ExternalInput")
    wT = nc.dram_tensor("wT", [D, O], dt_mm, kind="ExternalInput")
    rc = nc.dram_tensor("rc", [D, ER], dt_mm, kind="ExternalInput")
    rcl = nc.dram_tensor("rcl", [D, E], F32, kind="ExternalInput")
    bc = nc.dram_tensor("bc", [ER + 1, O], dt_mm, kind="ExternalInput")
    y = nc.dram_tensor("y", [TOK, O], F32, kind="ExternalOutput")

    with tile.TileContext(nc) as tc:
        with (
            tc.tile_pool(name="const", bufs=1) as const,
            tc.tile_pool(name="xpool", bufs=1) as xpool,
            tc.tile_pool(name="wpool", bufs=2) as wpool,
            tc.tile_pool(name="small", bufs=2) as small,
            tc.tile_pool(name="whtp", bufs=1) as whtp,
            tc.tile_pool(name="opool", bufs=4) as opool,
            tc.tile_pool(name="pso", bufs=5, space="PSUM") as pso,
            tc.tile_pool(name="psaux", bufs=2, space="PSUM") as psaux,
        ):
            ident = const.tile([128, 128], F32)
            make_identity(nc, ident[:])

            # router+A params, transposed-stationary: rc_sb[p, k*RC+c]
            rcl_sb = const.tile([128, KC * E], F32)
            nc.scalar.dma_start(
                out=rcl_sb[:].rearrange("p (k c) -> p k c", k=KC),
                in_=rcl[:, :].rearrange("(k p) c -> p k c", p=128),
            )
            rc_sb = const.tile([128, KC * ER], dt_mm)
            KH = KC // 4
            for h in range(4):
                nc.scalar.dma_start(
                    out=rc_sb[:, h * KH * ER:(h + 1) * KH * ER].rearrange(
                        "p (k c) -> p k c", k=KH
                    ),
                    in_=rc[h * KH * 128:(h + 1) * KH * 128, :].rearrange(
                        "(k p) c -> p k c", p=128
                    ),
                )

            # x resident: one big tile, xbig[p, k*TOK + tok] = xT[k*128+p, tok]
            # W for oc0 interleaved so pass-2 oc0 can start while x streams.
            xbig = xpool.tile([128, KC * TOK], dt_mm, name="xbig")
            KQ = KC // 4  # 4 k-blocks per W DMA
            # x k-block DMA sizes: small first so prT matmuls start early
            xsizes = [1, 1, 2, 2, 2, 2, 2, 2, 2]
            w0_after = {4: 0, 8: 1, 12: 2, 16: 3}  # k0 -> w0 quarter to issue
            w0ks = []
            k0 = 0
            for sz in xsizes:
                nc.sync.dma_start(
                    out=xbig[:, k0 * TOK:(k0 + sz) * TOK].rearrange(
                        "p (k c) -> p k c", k=sz
                    ),
                    in_=xT[k0 * 128:(k0 + sz) * 128, :].rearrange(
                        "(k p) c -> p k c", p=128
                    ),
                )
                k0 += sz
                if k0 in w0_after:
                    q = w0_after[k0]
                    wq = wpool.tile(
                        [128, KQ * OCH], dt_mm, tag=f"w{q}",
                        bufs=3, name=f"w0_{q}"
                    )
                    nc.sync.dma_start(
                        out=wq[:].rearrange("p (k c) -> p k c", k=KQ),
                        in_=wT[q * KQ * 128:(q + 1) * KQ * 128, 0:OCH].rearrange(
                            "(k p) c -> p k c", p=128
                        ),
                    )
                    w0ks.append(wq)

            bc_sb = const.tile([ER + 1, O], dt_mm)
            nc.scalar.dma_start(out=bc_sb[:], in_=bc[:, :])

            # queue the rest of W now: deep DMA lookahead keeps the SP ring busy
            wks_all = [w0ks]
            for oc in range(1, NOC):
                wks = []
                for q in range(4):
                    wq = wpool.tile(
                        [128, KQ * OCH], dt_mm, tag=f"w{q}",
                        bufs=3, name=f"w{oc}_{q}"
                    )
                    nc.sync.dma_start(
                        out=wq[:].rearrange("p (k c) -> p k c", k=KQ),
                        in_=wT[q * KQ * 128:(q + 1) * KQ * 128,
                               oc * OCH:(oc + 1) * OCH].rearrange(
                            "(k p) c -> p k c", p=128
                        ),
                    )
                    wks.append(wq)
                wks_all.append(wks)

            # ---- pass 1a: hT = A.T @ x -> [64, 512] per chunk (f32r),
            #      logits token-major in f32 (exact top-k)
            prTs = []
            for tcn in range(TC2):
                prT = psaux.tile([ER, 512], F32, tag="big", name=f"prT{tcn}")
                for k in range(KC):
                    nc.tensor.matmul(
                        prT[:],
                        rc_sb[:, k * ER:(k + 1) * ER],
                        xbig[:, k * TOK + tcn * 512:k * TOK + tcn * 512 + 512],
                        start=(k == 0),
                        stop=(k == KC - 1),
                    )
                prTs.append(prT)

            # hT in sbuf (f32): rows e*8+j, cols = token; row 64 = ones (bias)
            hT = const.tile([ER + 1, TOK], F32)
            nc.vector.memset(hT[ER:ER + 1, :], 1.0)
            for tcn in range(TC2):
                nc.vector.tensor_copy(
                    hT[0:ER, tcn * 512:(tcn + 1) * 512], prTs[tcn][0:ER, :]
                )

            # logits: 8 groups (one per token tile) into one PSUM bank, f32
            plg = psaux.tile([128, NT * E], F32, tag="plg", bufs=1, name="plg")
            for t in range(NT):
                for k in range(KC):
                    nc.tensor.matmul(
                        plg[:, t * E:(t + 1) * E],
                        xbig[:, k * TOK + t * 128:k * TOK + t * 128 + 128]
                        .bitcast(F32),
                        rcl_sb[:, k * E:(k + 1) * E],
                        start=(k == 0),
                        stop=(k == KC - 1),
                    )
            LG = small.tile([128, NT * E], F32, tag="LG", name="LG")
            nc.vector.tensor_copy(LG[:], plg[:])

            # ---- pass 1b: batched top-2 softmax gates over all NT tiles ----
            LG3 = LG[:].rearrange("p (t e) -> p t e", t=NT)
            m1 = small.tile([128, NT], F32, tag="m1", name="m1")
            nc.vector.reduce_max(m1[:], LG3, axis=mybir.AxisListType.X)
            selmax = small.tile([128, NT * E], F32, tag="selmax", name="selmax")
            nc.vector.tensor_tensor(
                out=selmax[:].rearrange("p (t e) -> p t e", t=NT),
                in0=LG3,
                in1=m1[:].unsqueeze(-1).broadcast_to([128, NT, E]),
                op=mybir.AluOpType.is_ge,
            )
            masked = small.tile([128, NT * E], F32, tag="masked", name="masked")
            nc.vector.scalar_tensor_tensor(
                out=masked[:], in0=selmax[:], scalar=-1e30, in1=LG[:],
                op0=mybir.AluOpType.mult, op1=mybir.AluOpType.add,
            )
            m2 = small.tile([128, NT], F32, tag="m2", name="m2")
            nc.vector.reduce_max(
                m2[:], masked[:].rearrange("p (t e) -> p t e", t=NT),
                axis=mybir.AxisListType.X,
            )
            d1 = small.tile([128, NT * E], F32, tag="d1", name="d1")
            nc.vector.tensor_tensor(
                out=d1[:].rearrange("p (t e) -> p t e", t=NT),
                in0=LG3,
                in1=m1[:].unsqueeze(-1).broadcast_to([128, NT, E]),
                op=mybir.AluOpType.subtract,
            )
            eall = small.tile([128, NT * E], F32, tag="eall", name="eall")
            nc.scalar.activation(
                eall[:], d1[:], mybir.ActivationFunctionType.Exp,
            )
            d2 = small.tile([128, NT], F32, tag="d2", name="d2")
            nc.vector.tensor_sub(d2[:], m2[:], m1[:])
            e2 = small.tile([128, NT], F32, tag="e2", name="e2")
            nc.scalar.activation(
                e2[:], d2[:], mybir.ActivationFunctionType.Exp,
            )
            denom = small.tile([128, NT], F32, tag="denom", name="denom")
            nc.vector.tensor_scalar(
                out=denom[:], in0=e2[:], scalar1=1.0, scalar2=None,
                op0=mybir.AluOpType.add,
            )
            invd = small.tile([128, NT], F32, tag="invd", name="invd")
            nc.vector.reciprocal(invd[:], denom[:])
            sel = small.tile([128, NT * E], F32, tag="sel", name="sel")
            nc.vector.tensor_tensor(
                out=sel[:].rearrange("p (t e) -> p t e", t=NT),
                in0=LG3,
                in1=m2[:].unsqueeze(-1).broadcast_to([128, NT, E]),
                op=mybir.AluOpType.is_ge,
            )
            gsel = small.tile([128, NT * E], F32, tag="gsel", name="gsel")
            nc.vector.tensor_mul(gsel[:], eall[:], sel[:])
            ginv = small.tile([128, NT * E], F32, tag="ginv", name="ginv")
            nc.vector.tensor_tensor(
                out=ginv[:].rearrange("p (t e) -> p t e", t=NT),
                in0=gsel[:].rearrange("p (t e) -> p t e", t=NT),
                in1=invd[:].unsqueeze(-1).broadcast_to([128, NT, E]),
                op=mybir.AluOpType.mult,
            )

            # g_rep[t]: [128, 64] with col e*8+j = gate[tok, e]
            greps = []
            for t in range(NT):
                grep = small.tile(
                    [128, ER + 1], F32, tag=f"grep{t}", name=f"grep{t}"
                )
                nc.vector.tensor_copy(
                    grep[:, 0:ER].rearrange("p (e r) -> p e r", e=E),
                    ginv[:, t * E:(t + 1) * E].unsqueeze(-1).broadcast_to(
                        [128, E, R]
                    ),
                )
                nc.vector.memset(grep[:, ER:ER + 1], 1.0)
                greps.append(grep)
            whts = []
            for half in range(2):
                gtp = psaux.tile(
                    [ER + 1, 4 * 128], F32, tag="big", name=f"gtp{half}"
                )
                for i in range(4):
                    t = half * 4 + i
                    nc.tensor.transpose(
                        gtp[:, i * 128:(i + 1) * 128], greps[t][:], ident[:]
                    )
                for i in range(4):
                    t = half * 4 + i
                    wht = whtp.tile(
                        [ER + 1, 128], dt_mm, tag=f"wht{t}", name=f"wht{t}"
                    )
                    nc.vector.tensor_mul(
                        wht[:], hT[:, t * 128:(t + 1) * 128],
                        gtp[:, i * 128:(i + 1) * 128],
                    )
                    whts.append(wht)

            # ---- pass 2: base matmul + LoRA combine + bias ----
            KQ = KC // 4
            for oc in range(NOC):
                wks = wks_all[oc]
                for t in range(NT):
                    po = pso.tile([128, OCH], F32, tag="po", name=f"po{oc}_{t}")
                    for k in range(KC):
                        nc.tensor.matmul(
                            po[:],
                            xbig[:, k * TOK + t * 128:k * TOK + t * 128 + 128],
                            wks[k // KQ][:, (k % KQ) * OCH:(k % KQ + 1) * OCH],
                            start=(k == 0),
                            stop=False,
                        )
                    nc.tensor.matmul(
                        po[:],
                        whts[t][:],
                        bc_sb[:, oc * OCH:(oc + 1) * OCH],
                        start=False,
                        stop=True,
                    )
                    ot = opool.tile([128, OCH], F32, tag="ot", name=f"ot{oc}_{t}")
                    if (oc * NT + t) % 2 == 0:
                        nc.vector.tensor_copy(ot[:], po[:])
                    else:
                        nc.scalar.copy(ot[:], po[:])
                    nc.scalar.dma_start(
                        out=y[t * 128:(t + 1) * 128, oc * OCH:(oc + 1) * OCH],
                        in_=ot[:],
                    )
    nc.compile()
    _CACHE[key] = nc
    return nc


def _prep_shared(W_base, b_base, W_router, A_w, B_w):
    wT = np.ascontiguousarray(W_base.T)                       # [D, O]
    rc = np.ascontiguousarray(
        A_w.transpose(2, 0, 1).reshape(D, ER).astype(np.float32)
    )                                                         # [D, 64]
    rcl = np.ascontiguousarray(W_router.T.astype(np.float32))  # [D, 8]
    bc = np.concatenate(
        [SCALE * B_w.transpose(0, 2, 1).reshape(ER, O), b_base[None, :]], axis=0
    ).astype(np.float32)                                      # [65, O]
    return np.ascontiguousarray(wT), rc, rcl, np.ascontiguousarray(bc)


def kernel(x, W_base, b_base, W_router, A_w, B_w, _trace=False):
    x = np.asarray(x, dtype=np.float32)
    W_base = np.asarray(W_base, dtype=np.float32)
    b_base = np.asarray(b_base, dtype=np.float32)
    W_router = np.asarray(W_router, dtype=np.float32)
    A_w = np.asarray(A_w, dtype=np.float32)
    B_w = np.asarray(B_w, dtype=np.float32)

    nc = _build_program()
    wT, rc, rcl, bc = _prep_shared(W_base, b_base, W_router, A_w, B_w)
    x_flat = x.reshape(NTOK, D)
    in_maps = []
    for i in range(NCORES):
        shard = x_flat[i * TOK:(i + 1) * TOK]
        in_maps.append({
            "xT": np.ascontiguousarray(shard.T),
            "wT": wT, "rc": rc, "rcl": rcl, "bc": bc,
        })
    res = run_bass_kernel_spmd(
        nc, in_maps, core_ids=list(range(NCORES)), trace=_trace,
    )
    out = np.concatenate([res.results[i]["y"] for i in range(NCORES)], axis=0)
    if _trace:
        kernel._last_results = res
    return out.reshape(B, S, O)



# revision 5
# speedup vs baseline: 1.4040x; 1.4040x over previous
"""MoLA adapter (MoE-of-LoRA) Trainium2 kernel — fp8 DoubleRow edition.

out = x @ W_base.T + b_base
      + sum_{e in top2(router(x))} softmax_gate_e * (x @ A_e.T @ B_e.T) * (alpha/r)

Strategy (8 NeuronCores, data-parallel over tokens; 1024 tokens/core):
  - All heavy matmuls run as fp8(e4m3) DoubleRow (2 contraction rows per
    partition, 0.5 cyc/row = 4x the f32r rate). Precision comes from a
    3-term split with shared product scale C = sx*sw = 8192:
        x @ W ~= x8@W8 + dx8@W8 + x8@dW8
    where x8 = fp8(16*x), dx8 = fp8(16*(x - x8/16)), W8 = fp8(512*W),
    dW8 = fp8(512*(W - W8/512)). Measured end-to-end rel err ~4.5e-3.
  - Router logits also DoubleRow fp8: moving operand packs [Wr_hi | Wr_lo]
    (two fp8 words per router weight) so logits carry ~1e-3 noise; top-2 +
    softmax-of-2 gates computed on 16x-scaled logits (Exp applies 1/16).
  - LoRA-A (h = x@A.T) is a single fp8 DoubleRow pass; combine matmul
    (wh.T @ [SCALE*B | b_base]) is DoubleRow with K=65 packed as 2x64
    (row 64 = ones carries the bias; zero rows pad).
  - Base PSUM group per (oc, t) tile: 8+8+8 DoubleRow matmuls (3 passes)
    + 1 combine, dequantized by 1/8192 on the PSUM->SBUF copy.
  - All inputs host-packed into SBUF layout (contiguous >=512B runs):
    ~12.2 MB/core in + 8 MB out vs ~91 us PE busy -> PE-bound.
"""

import numpy as np
import ml_dtypes

import concourse.bass as bass
import concourse.mybir as mybir
import concourse.tile as tile
from concourse import bacc
from concourse.bass_utils import run_bass_kernel_spmd
from concourse.masks import make_identity

# problem dims (hardcoded per contract)
B, S, D, O = 4, 2048, 2048, 2048
E, R, TOPK = 8, 8, 2
SCALE = 16.0 / R  # alpha / r
NCORES = 8
NTOK = B * S
TOK = NTOK // NCORES        # tokens per core = 1024
NT = TOK // 128             # 8 token tiles per core
KC = D // 128               # 16 contraction chunks
KP = KC // 2                # 8 DoubleRow k-pairs
OCH = 512
NOC = O // OCH              # 4 output chunks
ER = E * R                  # 64

SX = 16.0                   # x fp8 scale
SW = 512.0                  # W / A / B / router fp8 scale
CINV = 1.0 / (SX * SW)      # dequant for base+combine PSUM
RLO = 32.0                  # router lo-residual extra scale (stays in e4m3 range)
LGE = CINV                  # Exp scale: merged logits carry SX*SW scaling

F32 = mybir.dt.float32
FP8 = mybir.dt.float8e4
DR = mybir.MatmulPerfMode.DoubleRow
E4M3 = ml_dtypes.float8_e4m3

_CACHE = {}


def _build_program():
    key = "prog"
    if key in _CACHE:
        return _CACHE[key]
    nc = bacc.Bacc("TRN2", target_bir_lowering=False, debug=False)
    TC2 = TOK // 512            # 2 chunks of 512 tokens for the prT matmul
    # host-packed SBUF-layout inputs (all fp8 except output)
    x8d = nc.dram_tensor("x8", [128, KC * TOK], FP8, kind="ExternalInput")
    dx8d = nc.dram_tensor("dx8", [128, KC * TOK], FP8, kind="ExternalInput")
    w8d = nc.dram_tensor("w8", [128, NOC * KC * OCH], FP8, kind="ExternalInput")
    dw8d = nc.dram_tensor("dw8", [128, NOC * KC * OCH], FP8, kind="ExternalInput")
    rc8d = nc.dram_tensor("rc8", [128, KC * ER], FP8, kind="ExternalInput")
    # router packed for DoubleRow: [p, kp*32 + i*16 + c], c = [hi(8) | lo(8)]
    rl8d = nc.dram_tensor("rl8", [128, KC * 16], FP8, kind="ExternalInput")
    bc8d = nc.dram_tensor("bc8", [64, 2 * O], FP8, kind="ExternalInput")
    y = nc.dram_tensor("y", [TOK, O], F32, kind="ExternalOutput")

    with tile.TileContext(nc) as tc:
        with (
            tc.tile_pool(name="const", bufs=1) as const,
            tc.tile_pool(name="xpool", bufs=1) as xpool,
            tc.tile_pool(name="wres", bufs=1) as wres,
            tc.tile_pool(name="small", bufs=2) as small,
            tc.tile_pool(name="whtp", bufs=1) as whtp,
            tc.tile_pool(name="opool", bufs=4) as opool,
            tc.tile_pool(name="pso", bufs=5, space="PSUM") as pso,
            tc.tile_pool(name="psaux", bufs=2, space="PSUM") as psaux,
        ):
            ident = const.tile([128, 128], F32)
            make_identity(nc, ident[:])

            # small constants on the ACT ring
            rc8_sb = const.tile([128, KC * ER], FP8)
            nc.scalar.dma_start(out=rc8_sb[:], in_=rc8d[:, :])
            rl8_sb = const.tile([128, KC * 16], FP8)
            nc.scalar.dma_start(out=rl8_sb[:], in_=rl8d[:, :])
            bc8_sb = const.tile([64, 2 * O], FP8)
            nc.scalar.dma_start(out=bc8_sb[:], in_=bc8d[:, :])

            # x8 streamed in 2k-chunks on the SP ring (prT k-pairs chase)
            x8 = xpool.tile([128, KC * TOK], FP8, name="x8")
            for c in range(KP):
                nc.sync.dma_start(
                    out=x8[:, 2 * c * TOK:(2 * c + 2) * TOK],
                    in_=x8d[:, 2 * c * TOK:(2 * c + 2) * TOK],
                )
            # W8 oc0 next (base pass1 starts), then dx8, dw8 oc0, rest
            w8s, dw8s = [None] * NOC, [None] * NOC
            WCH = KC * OCH

            def load_w(lst, dram, oc, nm):
                t = wres.tile([128, WCH], FP8, tag=f"{nm}{oc}", name=f"{nm}{oc}")
                for h in range(2):
                    nc.sync.dma_start(
                        out=t[:, h * WCH // 2:(h + 1) * WCH // 2],
                        in_=dram[:, oc * WCH + h * WCH // 2:
                                 oc * WCH + (h + 1) * WCH // 2],
                    )
                lst[oc] = t

            load_w(w8s, w8d, 0, "w")
            dx8 = xpool.tile([128, KC * TOK], FP8, name="dx8")
            for c in range(4):
                nc.sync.dma_start(
                    out=dx8[:, 4 * c * TOK:(4 * c + 4) * TOK],
                    in_=dx8d[:, 4 * c * TOK:(4 * c + 4) * TOK],
                )
            load_w(dw8s, dw8d, 0, "dw")
            for oc in range(1, NOC):
                load_w(w8s, w8d, oc, "w")
                load_w(dw8s, dw8d, oc, "dw")

            def xpair(xt, kp, col0, ncol):
                # stationary/moving AP [128, 2, ncol] over token cols
                return xt[:, 2 * kp * TOK:(2 * kp + 2) * TOK].rearrange(
                    "p (i c) -> p i c", i=2
                )[:, :, col0:col0 + ncol]

            # ---- pass 1a: hT = A.T @ x -> [64, 512] per chunk, fp8 1-pass
            hT = const.tile([ER, TOK], F32)
            prTs = []
            for tcn in range(TC2):
                prT = psaux.tile([ER, 512], F32, tag="big", name=f"prT{tcn}")
                for kp in range(KP):
                    nc.tensor.matmul(
                        prT[:],
                        rc8_sb[:, 2 * kp * ER:(2 * kp + 2) * ER].rearrange(
                            "p (i c) -> p i c", i=2
                        ),
                        xpair(x8, kp, tcn * 512, 512),
                        start=(kp == 0),
                        stop=(kp == KP - 1),
                        perf_mode=DR,
                    )
                prTs.append(prT)
                nc.scalar.mul(
                    hT[:, tcn * 512:(tcn + 1) * 512], prT[:], CINV
                )

            # ---- logits, DoubleRow fp8, hi/lo router split, 2 passes ----
            # plgs[:, t*16 + (hi/lo)*8 + e], SX-scaled after the 1/SW merge
            plgs = psaux.tile([128, NT * 16], F32, tag="plg", bufs=1, name="plgs")
            for t in range(NT):
                for xi, xt in enumerate((x8, dx8)):
                    for kp in range(KP):
                        nc.tensor.matmul(
                            plgs[:, t * 16:(t + 1) * 16],
                            xpair(xt, kp, t * 128, 128),
                            rl8_sb[:, kp * 32:(kp + 1) * 32].rearrange(
                                "p (i c) -> p i c", i=2
                            ),
                            start=(xi == 0 and kp == 0),
                            stop=(xi == 1 and kp == KP - 1),
                            perf_mode=DR,
                        )
            # LG[p, t*8+e] = hi + lo/SW (only one PSUM operand per DVE op)
            P3 = plgs[:].rearrange("p (t i e) -> p t i e", t=NT, i=2)
            LGH = small.tile([128, NT * E], F32, tag="LGH", name="LGH")
            nc.vector.tensor_copy(
                LGH[:].rearrange("p (t e) -> p t e", t=NT), P3[:, :, 0, :]
            )
            LG = small.tile([128, NT * E], F32, tag="LG", name="LG")
            nc.vector.scalar_tensor_tensor(
                out=LG[:].rearrange("p (t e) -> p t e", t=NT),
                in0=P3[:, :, 1, :], scalar=1.0 / RLO,
                in1=LGH[:].rearrange("p (t e) -> p t e", t=NT),
                op0=mybir.AluOpType.mult, op1=mybir.AluOpType.add,
            )

            # ---- top-2 + softmax gates on SX-scaled logits ----
            LG3 = LG[:].rearrange("p (t e) -> p t e", t=NT)
            m1 = small.tile([128, NT], F32, tag="m1", name="m1")
            nc.vector.reduce_max(m1[:], LG3, axis=mybir.AxisListType.X)
            selmax = small.tile([128, NT * E], F32, tag="selmax", name="selmax")
            nc.vector.tensor_tensor(
                out=selmax[:].rearrange("p (t e) -> p t e", t=NT),
                in0=LG3,
                in1=m1[:].unsqueeze(-1).broadcast_to([128, NT, E]),
                op=mybir.AluOpType.is_ge,
            )
            masked = small.tile([128, NT * E], F32, tag="masked", name="masked")
            nc.vector.scalar_tensor_tensor(
                out=masked[:], in0=selmax[:], scalar=-1e30, in1=LG[:],
                op0=mybir.AluOpType.mult, op1=mybir.AluOpType.add,
            )
            m2 = small.tile([128, NT], F32, tag="m2", name="m2")
            nc.vector.reduce_max(
                m2[:], masked[:].rearrange("p (t e) -> p t e", t=NT),
                axis=mybir.AxisListType.X,
            )
            d1 = small.tile([128, NT * E], F32, tag="d1", name="d1")
            nc.vector.tensor_tensor(
                out=d1[:].rearrange("p (t e) -> p t e", t=NT),
                in0=LG3,
                in1=m1[:].unsqueeze(-1).broadcast_to([128, NT, E]),
                op=mybir.AluOpType.subtract,
            )
            eall = small.tile([128, NT * E], F32, tag="eall", name="eall")
            nc.scalar.activation(
                eall[:], d1[:], mybir.ActivationFunctionType.Exp, scale=LGE,
            )
            d2 = small.tile([128, NT], F32, tag="d2", name="d2")
            nc.vector.tensor_sub(d2[:], m2[:], m1[:])
            e2 = small.tile([128, NT], F32, tag="e2", name="e2")
            nc.scalar.activation(
                e2[:], d2[:], mybir.ActivationFunctionType.Exp, scale=LGE,
            )
            denom = small.tile([128, NT], F32, tag="denom", name="denom")
            nc.vector.tensor_scalar(
                out=denom[:], in0=e2[:], scalar1=1.0, scalar2=None,
                op0=mybir.AluOpType.add,
            )
            invd = small.tile([128, NT], F32, tag="invd", name="invd")
            nc.vector.reciprocal(invd[:], denom[:])
            sel = small.tile([128, NT * E], F32, tag="sel", name="sel")
            nc.vector.tensor_tensor(
                out=sel[:].rearrange("p (t e) -> p t e", t=NT),
                in0=LG3,
                in1=m2[:].unsqueeze(-1).broadcast_to([128, NT, E]),
                op=mybir.AluOpType.is_ge,
            )
            gsel = small.tile([128, NT * E], F32, tag="gsel", name="gsel")
            nc.vector.tensor_mul(gsel[:], eall[:], sel[:])
            ginv = small.tile([128, NT * E], F32, tag="ginv", name="ginv")
            nc.vector.tensor_tensor(
                out=ginv[:].rearrange("p (t e) -> p t e", t=NT),
                in0=gsel[:].rearrange("p (t e) -> p t e", t=NT),
                in1=invd[:].unsqueeze(-1).broadcast_to([128, NT, E]),
                op=mybir.AluOpType.mult,
            )

            # g_rep[t]: [128, 64] with col e*8+j = gate[tok, e]
            greps = []
            for t in range(NT):
                grep = small.tile(
                    [128, ER], F32, tag=f"grep{t}", name=f"grep{t}"
                )
                nc.vector.tensor_copy(
                    grep[:].rearrange("p (e r) -> p e r", e=E),
                    ginv[:, t * E:(t + 1) * E].unsqueeze(-1).broadcast_to(
                        [128, E, R]
                    ),
                )
                greps.append(grep)
            # whT8[t]: [64, 2*128] fp8 = [SX*g*h | bias-ones row packing]
            whts = []
            for half in range(2):
                gtp = psaux.tile(
                    [ER, 4 * 128], F32, tag="big", name=f"gtp{half}"
                )
                for i in range(4):
                    t = half * 4 + i
                    nc.tensor.transpose(
                        gtp[:, i * 128:(i + 1) * 128], greps[t][:], ident[:]
                    )
                for i in range(4):
                    t = half * 4 + i
                    wht = whtp.tile(
                        [64, 256], FP8, tag=f"wht{t}", name=f"wht{t}"
                    )
                    # group 0: rows 0..63 = SX * h * gate
                    nc.vector.scalar_tensor_tensor(
                        out=wht[:, 0:128],
                        in0=hT[:, t * 128:(t + 1) * 128], scalar=SX,
                        in1=gtp[:, i * 128:(i + 1) * 128],
                        op0=mybir.AluOpType.mult, op1=mybir.AluOpType.mult,
                    )
                    # group 1: row 64 (partition 0) = ones * SX; rest zero
                    nc.vector.memset(wht[:, 128:256], 0.0)
                    nc.vector.memset(wht[0:1, 128:256], SX)
                    whts.append(wht)

            # ---- pass 2: base (3 fp8 passes) + LoRA combine + bias ----
            bc3 = bc8_sb[:].rearrange("p (i c) -> p i c", i=2)
            for oc in range(NOC):
                w8t, dw8t = w8s[oc], dw8s[oc]
                w3 = w8t[:].rearrange("p (k c) -> p k c", k=KC)
                dw3 = dw8t[:].rearrange("p (k c) -> p k c", k=KC)
                for t in range(NT):
                    po = pso.tile([128, OCH], F32, tag="po", name=f"po{oc}_{t}")
                    # pass1: x8 . W8 ; pass3: x8 . dW8 ; pass2: dx8 . W8
                    # (order: 1,3 first chase w/dw DMAs; 2 needs dx8)
                    for pi, (xt, wt3) in enumerate(
                        ((x8, w3), (x8, dw3), (dx8, w3))
                    ):
                        for kp in range(KP):
                            nc.tensor.matmul(
                                po[:],
                                xpair(xt, kp, t * 128, 128),
                                wt3[:, 2 * kp:2 * kp + 2, :],
                                start=(pi == 0 and kp == 0),
                                stop=False,
                                perf_mode=DR,
                            )
                    nc.tensor.matmul(
                        po[:],
                        whts[t][:].rearrange("p (i c) -> p i c", i=2),
                        bc3[:, :, oc * OCH:(oc + 1) * OCH],
                        start=False,
                        stop=True,
                        perf_mode=DR,
                    )
                    ot = opool.tile([128, OCH], F32, tag="ot", name=f"ot{oc}_{t}")
                    if (oc * NT + t) % 2 == 0:
                        nc.vector.tensor_scalar(
                            out=ot[:], in0=po[:], scalar1=CINV, scalar2=None,
                            op0=mybir.AluOpType.mult,
                        )
                    else:
                        nc.scalar.mul(ot[:], po[:], CINV)
                    nc.scalar.dma_start(
                        out=y[t * 128:(t + 1) * 128, oc * OCH:(oc + 1) * OCH],
                        in_=ot[:],
                    )
    nc.compile()
    _CACHE[key] = nc
    return nc


def _q8(a, s):
    return np.asarray(a * s, dtype=np.float32).astype(E4M3)


def _prep_shared(W_base, b_base, W_router, A_w, B_w):
    # W packed [p, oc*KC*OCH + k*OCH + o] = Wt[k*128+p, oc*512+o]
    wt = np.ascontiguousarray(W_base.T)                        # [D, O]
    w8 = _q8(wt, SW)
    dw8 = _q8(wt - w8.astype(np.float32) / SW, SW)

    def packw(w):  # [D, O] -> [128, NOC*KC*OCH]
        a = w.reshape(KC, 128, NOC, OCH).transpose(1, 2, 0, 3)
        return np.ascontiguousarray(a.reshape(128, NOC * KC * OCH))

    # rc8[p, k*64 + c] = A_cat[k*128+p, c]
    acat = A_w.transpose(2, 0, 1).reshape(D, ER)
    rc8 = _q8(acat, SW).reshape(KC, 128, ER).transpose(1, 0, 2)
    rc8 = np.ascontiguousarray(rc8.reshape(128, KC * ER))
    # router hi/lo: rl8[p, kp*32 + i*16 + (hi|lo)] ; i = pair row
    wr = W_router.T.astype(np.float32)                         # [D, E]
    rh = _q8(wr, SW)
    rl = _q8(wr - rh.astype(np.float32) / SW, SW * RLO)
    rpack = np.concatenate(
        [rh.astype(np.float32), rl.astype(np.float32)], axis=1
    ).astype(E4M3)                                             # [D, 16]
    rl8 = rpack.reshape(KC, 128, 16).transpose(1, 0, 2)
    rl8 = np.ascontiguousarray(rl8.reshape(128, KC * 16))
    # bc8[p, i*O + o] = bc[i*64+p, o] ; bc rows 0..63 = SCALE*B, 64 = bias
    bc = np.concatenate(
        [SCALE * B_w.transpose(0, 2, 1).reshape(ER, O), b_base[None, :],
         np.zeros((63, O), np.float32)], axis=0
    )                                                          # [128, O]
    bc8 = _q8(bc, SW).reshape(2, 64, O).transpose(1, 0, 2)
    bc8 = np.ascontiguousarray(bc8.reshape(64, 2 * O))
    return packw(w8), packw(dw8), rc8, rl8, bc8


def _pack_x(shard):
    # [TOK, D] -> x8/dx8 [128, KC*TOK] fp8 with [p, k*TOK + m] layout
    xt = np.ascontiguousarray(shard.T)                         # [D, TOK]
    x8 = _q8(xt, SX)
    dx8 = _q8(xt - x8.astype(np.float32) / SX, SX)

    def pack(a):
        return np.ascontiguousarray(
            a.reshape(KC, 128, TOK).transpose(1, 0, 2).reshape(128, KC * TOK)
        )

    return pack(x8), pack(dx8)


def kernel(x, W_base, b_base, W_router, A_w, B_w, _trace=False):
    x = np.asarray(x, dtype=np.float32)
    W_base = np.asarray(W_base, dtype=np.float32)
    b_base = np.asarray(b_base, dtype=np.float32)
    W_router = np.asarray(W_router, dtype=np.float32)
    A_w = np.asarray(A_w, dtype=np.float32)
    B_w = np.asarray(B_w, dtype=np.float32)

    nc = _build_program()
    w8, dw8, rc8, rl8, bc8 = _prep_shared(W_base, b_base, W_router, A_w, B_w)
    x_flat = x.reshape(NTOK, D)
    in_maps = []
    for i in range(NCORES):
        x8, dx8 = _pack_x(x_flat[i * TOK:(i + 1) * TOK])
        in_maps.append({
            "x8": x8, "dx8": dx8, "w8": w8, "dw8": dw8,
            "rc8": rc8, "rl8": rl8, "bc8": bc8,
        })
    res = run_bass_kernel_spmd(
        nc, in_maps, core_ids=list(range(NCORES)), trace=_trace,
    )
    out = np.concatenate([res.results[i]["y"] for i in range(NCORES)], axis=0)
    if _trace:
        kernel._last_results = res
    return out.reshape(B, S, O)
